# revision 67
# baseline (speedup 1.0000x reference)
"""Transformer decoder layer (self-attn + cross-attn + FFN, pre-LN) on 8 trn2
NeuronCores.

Sharding: core = (batch b in 0..3) x (query-half h in {0,1}); every core
computes its 512 rows of all three outputs end-to-end (no collectives).
One uniform SPMD program; all per-core differences are carried in the data
(row slices, mask tiles).

Layout: activations are kept feature-major ([D on partitions, tokens on the
free dim]) so every matmul uses a weight (or activation) tile as the
stationary operand with no on-chip transposes of the trunk. Host pre-
transposes/casts inputs and pre-casts weights to bf16 (wq pre-scaled by
1/sqrt(dh), LN gammas folded into the consuming weights). Matmuls run in
bf16 with fp32 PSUM accumulation; the residual trunk stays fp32.

LayerNorm is computed as stats only (mean/var via 1/D-scaled ones-matmuls);
the normalize step is folded into each consuming projection as a K=2 rank-1
matmul (lhsT = [colsum; b@W] rows, rhs = [-mu; sd] rows) plus an rstd
multiply fused into the PSUM->SBUF copy-out. The FFN instead normalizes
x3 explicitly once (D=1024 chunks) because D_FF=4096 would need 4x the
copy-out multiplies.

Attention: scores^T = K^T tile-stationary x Q moving, two k-tiles share one
2-bank PSUM group so exp runs as a single [128,2,512] activation; softmax
uses exp without max subtraction; masking is a bf16 multiply with host mask
tiles; denominators come from a ones column appended to each head's V
slice; ctx^T is normalized via a rank-2 broadcast of the head-pair
reciprocals. Emission is software-pipelined: scores of pair p+1 are emitted
before ctx of pair p, and chunks of the (independent) cross-attention K/V
projections are interleaved between self-attention heads so the PE never
starves while the Act engine runs exp. head-0 probabilities and
denominators are DMA'd raw; the host transposes/divides/zero-fills.
"""

import numpy as np
import ml_dtypes
from contextlib import ExitStack

import concourse.bass as bass
import concourse.bacc as bacc
import concourse.tile as tile
import concourse.mybir as mybir
from concourse.bass_utils import run_bass_kernel_spmd
from concourse.masks import make_identity

bf16 = ml_dtypes.bfloat16
F32 = mybir.dt.float32
BF = mybir.dt.bfloat16
AF = mybir.ActivationFunctionType
ALU = mybir.AluOpType

B, L, D, H, DH, DFF = 4, 1024, 1024, 16, 64, 4096
LO = 512          # rows (query tokens) owned per core
DC = D // 128     # 8 feature chunks
FC = DFF // 128   # 32 ffn chunks
N_CORES = 8


class _Pools:
    pass


# ---------------------------------------------------------------- emission

def _ln_stats(tc, pools, x, Lt, pst, bcp, rows, rbp):
    """LN statistics in feature-major layout.

    x: list of 8 bf16 tiles [128, Lt]. Returns per-j (j = 512-token tile):
    pair tiles [2, 512] bf16 (row0 = -mu, row1 = sd), rstd rows [1,512] f32,
    and rstd broadcast tiles [128, 512] f32 in SBUF.
    """
    nc = tc.nc
    nt = Lt // 512
    pairs, rstds, rbs = [], [], []
    for j in range(nt):
        sl = slice(j * 512, (j + 1) * 512)
        ps_s = pst.tile([1, 512], F32, tag="st", name="st")
        ps_q = pst.tile([1, 512], F32, tag="st", name="st")
        for c in range(DC):
            nc.tensor.matmul(ps_s, pools.ones_bf, x[c][:, sl],
                             start=(c == 0), stop=(c == DC - 1))
        for c in range(DC):
            sq = pools.tmp.tile([128, 512], BF, tag="sq", name="sq", bufs=3)
            nc.vector.tensor_mul(sq, x[c][:, sl], x[c][:, sl])
            nc.tensor.matmul(ps_q, pools.ones_bf, sq,
                             start=(c == 0), stop=(c == DC - 1))
        pair = pools.pairp.tile([2, 512], BF, tag="pair", name="pair")
        nc.scalar.mul(pair[0:1, :], ps_s, -1.0)          # -mu (bf16)
        musq = rows.tile([1, 512], F32, tag="r", name="r")
        nc.vector.tensor_mul(musq, pair[0:1, :], pair[0:1, :])
        var = rows.tile([1, 512], F32, tag="r", name="r")
        nc.vector.tensor_sub(var, ps_q, musq)
        sd = rows.tile([1, 512], F32, tag="r", name="r")
        nc.scalar.activation(sd, var, AF.Sqrt, bias=pools.eps_t)
        sd_bf = rows.tile([1, 512], BF, tag="rb2", name="rb2", bufs=2)
        nc.vector.tensor_copy(sd_bf, sd)
        nc.sync.dma_start(out=pair[1:2, :], in_=sd_bf)   # sd into partition 1
        rstd = rows.tile([1, 512], BF, tag="rbf", name="rbf", bufs=2)
        with nc.allow_low_precision(reason="bf16 rstd: 0.4% on LN scale is fine"):
            nc.vector.reciprocal(rstd, sd)
        bc = bcp.tile([128, 512], F32, tag="bc", name="bc")
        nc.tensor.matmul(bc, pools.onesr_bf, rstd, start=True, stop=True)
        rb = rbp.tile([128, 512], F32, tag="rb", name="rb")
        nc.vector.tensor_copy(rb, bc)
        pairs.append(pair)
        rstds.append(rstd)
        rbs.append(rb)
    return pairs, rstds, rbs


def _proj_fm(tc, pools, w_dram, rhs, Lt, out_tiles=None, writer=None, fold=None):
    """out^T[oc] = w^T @ rhs  (feature-major result).

    rhs: 8 bf16 tiles [128, Lt] (feature-major activations; raw x when fold
    is set). fold = (fold_tile [2,1024] bf16, pairs per j, rbs per j): adds
    the rank-1 LN correction and scales by rstd at copy-out.
    w_dram is host-shuffled to [p, oc, ic, n] so each oc-tile DMA reads a
    contiguous 2KB run per partition.
    """
    nc = tc.nc
    nt = Lt // 512
    w_view = w_dram.ap().rearrange("p (oc ic n) -> p oc ic n", oc=DC, ic=DC)
    with tc.tile_pool(name="pp", bufs=4, space="PSUM") as pp:
        for oc in range(DC):
            wt = pools.wpool.tile([128, DC, 128], BF, tag="w", name="w")
            nc.sync.dma_start(out=wt, in_=w_view[:, oc])
            for j in range(nt):
                sl = slice(j * 512, (j + 1) * 512)
                ps = pp.tile([128, 512], F32, tag="ps", name="ps")
                last_plain = fold is None
                for ic in range(DC):
                    nc.tensor.matmul(ps, wt[:, ic, :], rhs[ic][:, sl],
                                     start=(ic == 0),
                                     stop=(last_plain and ic == DC - 1))
                if fold is not None:
                    ft, pairs, rbs = fold
                    nc.tensor.matmul(ps, ft[:, oc * 128:(oc + 1) * 128], pairs[j],
                                     start=False, stop=True)
                    nc.vector.tensor_mul(out_tiles[oc][:, sl], ps, rbs[j])
                elif writer is not None:
                    writer(oc, j, ps)
                else:
                    nc.vector.tensor_copy(out_tiles[oc][:, sl], ps)


def _proj_v_chunk(tc, pools, wvp, pp, wv_view, lhsT, vb, half, lt, vfold, rstdT,
                  wvt_cache):
    """One (half, lt) chunk of the token-major V projection."""
    nc = tc.nc
    if wvt_cache.get(half) is None:
        wvt = wvp.tile([128, DC, 512], BF, tag="wv", name="wv")
        nc.sync.dma_start(out=wvt, in_=wv_view[:, half])
        wvt_cache[half] = wvt
    wvt = wvt_cache[half]
    tsl = slice((lt % 4) * 128, (lt % 4) * 128 + 128)
    ps = pp.tile([128, 512], F32, tag="vps", name="vps")
    for ic in range(DC):
        nc.tensor.matmul(ps, lhsT[ic][:, lt * 128:(lt + 1) * 128],
                         wvt[:, ic, :],
                         start=(ic == 0), stop=(vfold is None and ic == DC - 1))
    dst = vb[lt].rearrange("p (h c) -> p h c", c=65)
    psv = ps.rearrange("p (h c) -> p h c", c=64)
    if vfold is not None:
        vft, pairs = vfold
        nc.tensor.matmul(ps, pairs[lt // 4][:, tsl],
                         vft[:, half * 512:(half + 1) * 512],
                         start=False, stop=True)
        nc.vector.tensor_scalar(
            dst[:, half * 8:(half + 1) * 8, 0:64], psv,
            rstdT[lt], None, op0=ALU.mult)
    else:
        nc.vector.tensor_copy(
            dst[:, half * 8:(half + 1) * 8, 0:64], psv)


def _make_vb(tc, vpool):
    nc = tc.nc
    vb = []
    for lt in range(8):
        v = vpool.tile([128, H * 65], BF, tag="vb", name="vb")
        nc.vector.memset(v.rearrange("p (h c) -> p h c", c=65)[:, :, 64:65], 1.0)
        vb.append(v)
    return vb


def _attention(tc, pools, QT, KT, vb, masks, top_dram, topd_dram, ctxn,
               fillers, causal=False):
    """scores^T -> exp -> (mask) -> ctx^T via ones-augmented V -> normalize.

    Software-pipelined: scores/exp of k-pair p+1 are emitted before the ctx
    matmuls of pair p; after each head one pending filler (independent PE
    work) is emitted. Head-0 probabilities + denominators go to DRAM raw.

    causal: query columns are 64-token slots in ascending-cost order (slot s
    needs key tiles 0..s), so k-tile t only covers the column suffix
    [64t, 512) and only its leading 64 columns (the diagonal slot) need the
    mask; skipped regions are reconstructed as zeros on the host.
    """
    nc = tc.nc
    fillers = list(fillers)

    def fill():
        if fillers:
            fillers.pop(0)()

    def q0(k):
        return 64 * k if causal else 0

    with (
        tc.tile_pool(name="psc", bufs=2, space="PSUM") as psc,
        tc.tile_pool(name="pctx", bufs=2, space="PSUM") as pctx,
        tc.tile_pool(name="prex", bufs=1, space="PSUM") as prex,
        tc.tile_pool(name="Pp", bufs=5) as Pp,
        tc.tile_pool(name="arow", bufs=2) as rows,
        tc.tile_pool(name="ctxu", bufs=8) as ctxup,
    ):
        ctxu = [None] * DC
        # all head-pair reciprocals in one [2, 8*512] tile so normalization
        # can be deferred past the whole head loop (no PE stall on DVE)
        rp = rows.tile([2, DC * 512], BF, tag="rp", name="rp", bufs=1)
        for h in range(H):
            c, odd = h // 2, h % 2
            prow = slice(odd * 64, odd * 64 + 64)
            Pm = []

            def mk_pair(p, h=h, c=c, prow=prow):
                ps = psc.tile([128, 2, 512], F32, tag="sc", name="sc")
                pe = Pp.tile([128, 2, 512], BF, tag="P", name="P")
                for j in range(2):
                    k = 2 * p + j
                    o = q0(k)
                    nc.tensor.matmul(ps[:, j, o:512],
                                     KT[c][prow, k * 128:(k + 1) * 128],
                                     QT[c][prow, o:512], start=True, stop=True)
                if causal:
                    for j in range(2):
                        k = 2 * p + j
                        o = q0(k)
                        nc.scalar.activation(pe[:, j, o:512], ps[:, j, o:512],
                                             AF.Exp)
                        nc.vector.tensor_mul(pe[:, j, o:o + 64],
                                             pe[:, j, o:o + 64], masks[k])
                    return pe
                nc.scalar.activation(pe, ps, AF.Exp)
                if masks is not None:
                    pm = Pp.tile([128, 2, 512], BF, tag="P", name="P")
                    nc.vector.tensor_mul(pm, pe, masks[p])
                    return pm
                return pe

            cps = pctx.tile([65, 512], F32, tag="ctx", name="ctx")

            def mk_ctx(p, pm, cps=cps, h=h):
                for j in range(2):
                    k = 2 * p + j
                    o = q0(k)
                    nc.tensor.matmul(cps[:, o:512] if o else cps,
                                     vb[k][:, h * 65:(h + 1) * 65],
                                     pm[:, j, o:512],
                                     start=(k == 0), stop=(k == 7),
                                     skip_group_check=causal and k > 0)

            # pipeline: scores(p) ... scores(p+1), ctx(p)
            prev = None
            for p in range(4):
                pm = mk_pair(p)
                Pm.append(pm)
                if prev is not None:
                    mk_ctx(prev[0], prev[1])
                prev = (p, pm)
            mk_ctx(prev[0], prev[1])

            if h == 0 and top_dram is not None:
                if causal:
                    for t in range(8):
                        o = q0(t)
                        nc.sync.dma_start(
                            out=top_dram.ap()[t * 128:(t + 1) * 128, o:512],
                            in_=Pm[t // 2][:, t % 2, o:512])
                else:
                    for p in range(4):
                        nc.sync.dma_start(
                            out=top_dram.ap()[p * 256:(p + 1) * 256, :]
                            .rearrange("(j p) q -> p j q", p=128),
                            in_=Pm[p])

            if not causal or h % 2 == 0:
                fill()
            if h == 0 and topd_dram is not None:
                d_row = rows.tile([1, 512], F32, tag="d", name="d")
                nc.vector.tensor_copy(d_row, cps[64:65, :])
                nc.sync.dma_start(out=topd_dram.ap(), in_=d_row)
            csl = slice(c * 512, (c + 1) * 512)
            with nc.allow_low_precision(reason="bf16 softmax denom recip"):
                if odd == 0:
                    nc.vector.reciprocal(rp[0:1, csl], cps[64:65, :])
                else:
                    rt = rows.tile([1, 512], BF, tag="rt", name="rt", bufs=2)
                    nc.vector.reciprocal(rt, cps[64:65, :])
                    nc.sync.dma_start(out=rp[1:2, csl], in_=rt)
            if odd == 0:
                ctxu[c] = ctxup.tile([128, 512], BF, tag="cu", name="cu")
            nc.vector.tensor_copy(ctxu[c][prow, :], cps[0:64, :])

        while fillers:
            fillers.pop(0)()
        for c in range(DC):
            # expand the head-pair reciprocals over their 64-row halves
            rexp = prex.tile([128, 512], F32, tag="rx", name="rx")
            nc.tensor.matmul(rexp, pools.sel01,
                             rp[:, c * 512:(c + 1) * 512],
                             start=True, stop=True)
            nc.vector.tensor_mul(ctxn[c], ctxu[c], rexp)


def _emit(ctx, tc, T):
    nc = tc.nc
    pools = _Pools()

    const = ctx.enter_context(tc.tile_pool(name="const", bufs=1))
    pools.ones_bf = const.tile([128, 1], BF)
    nc.vector.memset(pools.ones_bf, 1.0 / D)
    pools.ones_f = const.tile([1, 128], F32)
    nc.vector.memset(pools.ones_f, 1.0)
    pools.onesr_bf = const.tile([1, 128], BF)
    nc.vector.memset(pools.onesr_bf, 1.0)
    pools.sel01 = const.tile([2, 128], BF)
    nc.sync.dma_start(out=pools.sel01, in_=T["sel01"].ap())
    pools.eps_t = const.tile([1, 1], F32)
    nc.vector.memset(pools.eps_t, 1e-6)
    fb1 = const.tile([128, 32], F32)
    nc.sync.dma_start(out=fb1, in_=T["b_ff1"].ap())
    fb2 = const.tile([128, 8], F32)
    nc.sync.dma_start(out=fb2, in_=T["b_ff2"].ap())

    pools.wpool = ctx.enter_context(tc.tile_pool(name="wpool", bufs=3))
    pools.foldp = ctx.enter_context(tc.tile_pool(name="foldp", bufs=2))
    trunk = ctx.enter_context(tc.tile_pool(name="trunk", bufs=16))
    act512 = ctx.enter_context(tc.tile_pool(name="act512", bufs=16))
    tmp = ctx.enter_context(tc.tile_pool(name="gtmp", bufs=3))
    pools.tmp = tmp
    rbp = ctx.enter_context(tc.tile_pool(name="rbp", bufs=3))
    pools.pairp = ctx.enter_context(tc.tile_pool(name="pairp", bufs=4))

    def t512(n=8):
        return [act512.tile([128, 512], BF, tag="a512", name="a512") for _ in range(n)]

    def trunk_t(n=8):
        return [trunk.tile([128, 512], F32, tag="trunk", name="trunk") for _ in range(n)]

    def load_fold(name, w=2048):
        ft = pools.foldp.tile([2, w // 2], BF, tag="fold", name="fold")
        nc.sync.dma_start(out=ft, in_=T[name].ap())
        return ft

    with (
        tc.tile_pool(name="a1024", bufs=9) as a1024,
        tc.tile_pool(name="vbp", bufs=8) as vpool,
    ):
        def t1024(n=8):
            return [a1024.tile([128, 1024], BF, tag="a1024", name="a1024") for _ in range(n)]

        # ---------------- loads + LN1 stats ----------------
        xo_w = act512.tile([128, DC, LO], BF, tag="ow", name="ow", bufs=1)
        xo_v = T["xoT_bf"].ap().rearrange("(c p) l -> p c l", p=128)
        for g in range(4):
            nc.sync.dma_start(out=xo_w[:, 2 * g:2 * g + 2, :],
                              in_=xo_v[:, 2 * g:2 * g + 2, :])
        xo = [xo_w[:, c, :] for c in range(DC)]
        xa_w = a1024.tile([128, DC, L], BF, tag="aw", name="aw", bufs=1)
        xa_v = T["xaT_bf"].ap().rearrange("(c p) l -> p c l", p=128)
        nc.sync.dma_start(out=xa_w[:, 0:4, :], in_=xa_v[:, 0:4, :])
        nc.sync.dma_start(out=xa_w[:, 4:8, :], in_=xa_v[:, 4:8, :])
        xa = [xa_w[:, c, :] for c in range(DC)]
        enc_w = a1024.tile([128, DC, L], BF, tag="aw", name="aw", bufs=1)
        nc.sync.dma_start(out=enc_w,
                          in_=T["encT_bf"].ap().rearrange("(c p) l -> p c l", p=128))
        enc = [enc_w[:, c, :] for c in range(DC)]

        QT = t512()
        KT = t1024()
        KcT = [a1024.tile([128, 1024], BF, tag="kc", name="kc", bufs=8)
               for _ in range(8)]
        with (
            tc.tile_pool(name="pst", bufs=4, space="PSUM") as pst,
            tc.tile_pool(name="pbc", bufs=2, space="PSUM") as pbc,
            tc.tile_pool(name="lrow", bufs=4) as lrows,
            tc.tile_pool(name="ptt", bufs=1, space="PSUM") as ptt,
        ):
            pair_o, _, rb_o = _ln_stats(tc, pools, xo, LO, pst, pbc, lrows, rbp)
            pair_a, rstd_a, rb_a = _ln_stats(tc, pools, xa, L, pst, pbc, lrows, rbp)
            # token-major rstd for the V projection
            iden1 = lrows.tile([1, 1], BF, tag="i1", name="i1", bufs=1)
            nc.vector.memset(iden1, 1.0)
            rstdT = []
            for lt in range(8):
                rps = ptt.tile([128, 1], BF, tag="rT", name="rT")
                nc.tensor.transpose(
                    rps, rstd_a[lt // 4][0:1, (lt % 4) * 128:(lt % 4) * 128 + 128],
                    iden1)
                rsb = tmp.tile([128, 1], F32, tag="rTs", name="rTs", bufs=8)
                nc.vector.tensor_copy(rsb, rps)
                rstdT.append(rsb)

        # ---------------- self-attention projections ----------------
        _proj_fm(tc, pools, T["w_sa_q"], xo, LO, out_tiles=QT,
                 fold=(load_fold("fold_saq"), pair_o, rb_o))
        _proj_fm(tc, pools, T["w_sa_k"], xa, L, out_tiles=KT,
                 fold=(load_fold("fold_sak"), pair_a, rb_a))
        vb = _make_vb(tc, vpool)
        sav_fold = (load_fold("fold_sav"), pair_a)
        wv_sa_view = T["w_sa_v"].ap().rearrange("p (h ic n) -> p h ic n", h=2, ic=DC)
        with (
            tc.tile_pool(name="wv", bufs=2) as wvp,
            tc.tile_pool(name="vpp", bufs=4, space="PSUM") as vpp,
        ):
            cache_sa = {}
            for half in range(2):
                for lt in range(8):
                    _proj_v_chunk(tc, pools, wvp, vpp, wv_sa_view, xa, vb,
                                  half, lt, sav_fold, rstdT, cache_sa)

        # ---------------- SA attention || CA K projections ----------
        wv_ca_view = T["w_ca_v"].ap().rearrange("p (h ic n) -> p h ic n", h=2, ic=DC)
        wk_ca_view = T["w_ca_k"].ap().rearrange("p (oc ic n) -> p oc ic n",
                                                oc=DC, ic=DC)

        fill_stack = ExitStack()
        wvp_f = fill_stack.enter_context(tc.tile_pool(name="wvf", bufs=2))
        vpp_f = fill_stack.enter_context(
            tc.tile_pool(name="vppf", bufs=1, space="PSUM"))
        cache_ca = {}

        def mk_cak_filler(oc):
            def f():
                wt = pools.wpool.tile([128, DC, 128], BF, tag="w", name="w")
                nc.sync.dma_start(out=wt, in_=wk_ca_view[:, oc])
                for j in range(2):
                    sl = slice(j * 512, (j + 1) * 512)
                    ps = vpp_f.tile([128, 512], F32, tag="vps", name="vps")
                    for ic in range(DC):
                        nc.tensor.matmul(ps, wt[:, ic, :], enc[ic][:, sl],
                                         start=(ic == 0), stop=(ic == DC - 1))
                    nc.vector.tensor_copy(KcT[oc][:, sl], ps)
            return f

        # SA attention fillers (popped on even heads): the 8 CA-K chunks
        sa_fillers = [mk_cak_filler(oc) for oc in range(DC)]

        with tc.tile_pool(name="maskp", bufs=1) as maskp:
            mk_w = maskp.tile([128, 8, 64], BF, tag="m", name="m", bufs=1)
            nc.sync.dma_start(out=mk_w,
                              in_=T["maskd_bf"].ap().rearrange("p (k l) -> p k l", k=8))
            masks = [mk_w[:, k, :] for k in range(8)]
            ctxn = t512()
            _attention(tc, pools, QT, KT, vb, masks, T["sa_top"], T["sa_topd"],
                       ctxn, sa_fillers, causal=True)

        # O-projection + residual -> x2 (fp32 trunk); bf16 cast fused in
        x2T = trunk_t()
        x2bf = t512()

        def wr_sa_o(oc, j, ps):
            xot = tmp.tile([128, 512], F32, tag="xres", name="xres", bufs=2)
            nc.sync.dma_start(out=xot, in_=T["xoT_f32"].ap()[oc * 128:(oc + 1) * 128, :])
            nc.vector.tensor_add(x2T[oc], ps, xot)
            nc.vector.tensor_copy(x2bf[oc], x2T[oc])

        _proj_fm(tc, pools, T["w_sa_o"], ctxn, LO, writer=wr_sa_o)

        # ---------------- LN2 + CA query ----------------
        QcT = t512()
        with (
            tc.tile_pool(name="pst2", bufs=4, space="PSUM") as pst,
            tc.tile_pool(name="pbc2", bufs=2, space="PSUM") as pbc,
            tc.tile_pool(name="lrow2", bufs=4) as lrows,
        ):
            pair_2, _, rb_2 = _ln_stats(tc, pools, x2bf, LO, pst, pbc, lrows, rbp)
        _proj_fm(tc, pools, T["w_ca_q"], x2bf, LO, out_tiles=QcT,
                 fold=(load_fold("fold_caq"), pair_2, rb_2))

        # CA V: half 0 here (needed by CA heads 0-7), half 1 as CA fillers
        vbc = _make_vb(tc, vpool)

        def mk_cav_filler(half, lt):
            def f():
                _proj_v_chunk(tc, pools, wvp_f, vpp_f, wv_ca_view, enc, vbc,
                              half, lt, None, None, cache_ca)
            return f

        for lt in range(8):
            mk_cav_filler(0, lt)()
        ca_fillers = [mk_cav_filler(1, lt) for lt in range(8)]
        ctxc = t512()
        _attention(tc, pools, QcT, KcT, vbc, None, T["ca_top"], T["ca_topd"],
                   ctxc, ca_fillers)
        fill_stack.close()

        x3T = trunk_t()
        x3bf = t512()

        def wr_ca_o(oc, j, ps):
            nc.vector.tensor_add(x3T[oc], ps, x2T[oc])
            nc.vector.tensor_copy(x3bf[oc], x3T[oc])

        _proj_fm(tc, pools, T["w_ca_o"], ctxc, LO, writer=wr_ca_o)

    # ---------------- FFN ----------------

    with (
        tc.tile_pool(name="lrowf", bufs=4) as lrows,
        tc.tile_pool(name="h1p", bufs=FC) as h1p,
        tc.tile_pool(name="ftmp", bufs=2) as ftmp,
        tc.tile_pool(name="w2p", bufs=3) as w2p,
    ):
        with (
            tc.tile_pool(name="pstf", bufs=4, space="PSUM") as pst,
            tc.tile_pool(name="pbcf", bufs=2, space="PSUM") as pbc,
        ):
            pair_f, _, rb_f = _ln_stats(tc, pools, x3bf, LO, pst, pbc, lrows, rbp)
            # explicit xn3 = (x3 - mu) * rstd in bf16 (cheaper than folding
            # the rank-1 correction into all 32 FFN1 output chunks)
            mups = pbc.tile([128, 512], F32, tag="mu", name="mu")
            nc.tensor.matmul(mups, pools.onesr_bf, pair_f[0][0:1, :],
                             start=True, stop=True)
            # all-bf16 normalize chain: DVE runs at 2x on 2-byte operands,
            # halving the serial chain that gates FFN1
            mubf = ftmp.tile([128, 512], BF, tag="mubf", name="mubf", bufs=1)
            nc.vector.tensor_copy(mubf, mups)
            rbbf = ftmp.tile([128, 512], BF, tag="rbbf", name="rbbf", bufs=1)
            nc.vector.tensor_copy(rbbf, rb_f[0])
            xn3 = t512()
            for c in range(DC):
                xs = ftmp.tile([128, 512], BF, tag="xs", name="xs", bufs=3)
                nc.vector.tensor_add(xs, x3bf[c], mubf)
                nc.vector.tensor_mul(xn3[c], xs, rbbf)
        fpp = ctx.enter_context(tc.tile_pool(name="fpp", bufs=4, space="PSUM"))
        h1 = []
        w1_view = T["w_ff1"].ap().rearrange("p (oc ic n) -> p oc ic n",
                                            oc=FC, ic=DC)
        for oc in range(FC):
            wt = pools.wpool.tile([128, DC, 128], BF, tag="w", name="w")
            nc.sync.dma_start(out=wt, in_=w1_view[:, oc])
            ps = fpp.tile([128, 512], F32, tag="f1", name="f1")
            for ic in range(DC):
                nc.tensor.matmul(ps, wt[:, ic, :], xn3[ic],
                                 start=(ic == 0), stop=(ic == DC - 1))
            ht = h1p.tile([128, 512], BF, tag="h1", name="h1")
            nc.scalar.activation(ht, ps, AF.Relu, bias=fb1[:, oc:oc + 1])
            h1.append(ht)
        w2_view = T["w_ff2"].ap().rearrange("p (oc ic n) -> p oc ic n",
                                            oc=DC, ic=FC)
        for oc in range(DC):
            wt2h = []
            for hf in range(2):
                w = w2p.tile([128, FC // 2, 128], BF, tag="w2", name="w2",
                             bufs=3)
                nc.sync.dma_start(
                    out=w, in_=w2_view[:, oc, hf * 16:(hf + 1) * 16])
                wt2h.append(w)
            ps = fpp.tile([128, 512], F32, tag="f1", name="f2")
            for ic in range(FC):
                nc.tensor.matmul(ps, wt2h[ic // 16][:, ic % 16, :], h1[ic],
                                 start=(ic == 0), stop=(ic == FC - 1))
            t1 = ftmp.tile([128, 512], F32, tag="fo", name="fo", bufs=2)
            nc.vector.tensor_add(t1, ps, x3T[oc])
            xout = ftmp.tile([128, 512], F32, tag="fo2", name="fo2", bufs=2)
            nc.scalar.activation(xout, t1, AF.Identity, bias=fb2[:, oc:oc + 1])
            nc.sync.dma_start(out=T["outT"].ap()[oc * 128:(oc + 1) * 128, :], in_=xout)


# ---------------------------------------------------------------- build/run

_CACHE = {}


def _build(repeat=1):
    if repeat == 1 and "nc" in _CACHE:
        return _CACHE["nc"], _CACHE["T"]
    nc = bacc.Bacc("TRN2", target_bir_lowering=False, debug=False)
    T = {}

    def din(name, shape, dt):
        T[name] = nc.dram_tensor(name, shape, dt, kind="ExternalInput")

    def dout(name, shape, dt):
        T[name] = nc.dram_tensor(name, shape, dt, kind="ExternalOutput")

    din("xoT_f32", [D, LO], F32)
    din("xoT_bf", [D, LO], BF)
    din("xaT_bf", [D, L], BF)
    din("encT_bf", [D, L], BF)
    din("maskd_bf", [128, 8 * 64], BF)
    for w in ["w_sa_q", "w_sa_k", "w_sa_v", "w_sa_o",
              "w_ca_q", "w_ca_k", "w_ca_v", "w_ca_o"]:
        din(w, [128, D * DC], BF)
    din("w_ff1", [128, DFF * DC], BF)
    din("w_ff2", [128, FC * D], BF)
    for f in ["fold_saq", "fold_sak", "fold_sav", "fold_caq"]:
        din(f, [2, D], BF)
    din("b_ff1", [128, 32], F32)
    din("b_ff2", [128, 8], F32)
    din("sel01", [2, 128], BF)
    dout("outT", [D, LO], F32)
    dout("sa_top", [L, LO], BF)
    dout("ca_top", [L, LO], BF)
    dout("sa_topd", [1, LO], F32)
    dout("ca_topd", [1, LO], F32)

    with tile.TileContext(nc) as tc:
        for _rep in range(repeat):
            with ExitStack() as ctx:
                _emit(ctx, tc, T)
    nc.compile()
    if repeat == 1:
        _CACHE["nc"] = nc
        _CACHE["T"] = T
    return nc, T


def _col(v, n):
    return np.ascontiguousarray(np.asarray(v, np.float32).reshape(n, 128).T)


def _wshuf(w, oc, ic):
    """[ic*128, oc*128] -> [128, oc, ic, 128] flattened: per-oc-tile DMA reads
    a contiguous run per partition."""
    a = np.asarray(w, np.float32).reshape(ic, 128, oc, 128)
    return np.ascontiguousarray(
        a.transpose(1, 2, 0, 3).reshape(128, -1).astype(bf16))


def _wshuf_v(w):
    """[1024, 1024] -> [128, half, ic, 512] flattened (V projections)."""
    a = np.asarray(w, np.float32).reshape(DC, 128, 2, 512)
    return np.ascontiguousarray(
        a.transpose(1, 2, 0, 3).reshape(128, -1).astype(bf16))


def _perm(hh):
    return np.concatenate([64 * (2 * s + hh) + np.arange(64) for s in range(8)])


def _prep_in_maps(inputs):
    f = {k: np.asarray(v, np.float32) if np.asarray(v).dtype != np.bool_
         else np.asarray(v) for k, v in inputs.items()}
    common = {}

    def fold(wname, w_scaled, g, b):
        wg = np.ascontiguousarray(g[:, None] * w_scaled)
        common[wname] = _wshuf_v(wg) if wname.endswith("_v") else _wshuf(wg, DC, DC)
        cs = wg.sum(axis=0)
        ob = b @ w_scaled
        return np.ascontiguousarray(np.stack([cs, ob]).astype(bf16))

    common["fold_saq"] = fold("w_sa_q", f["sa_wq"] / 8.0, f["ln1_g"], f["ln1_b"])
    common["fold_sak"] = fold("w_sa_k", f["sa_wk"], f["ln1_g"], f["ln1_b"])
    common["fold_sav"] = fold("w_sa_v", f["sa_wv"], f["ln1_g"], f["ln1_b"])
    common["fold_caq"] = fold("w_ca_q", f["ca_wq"] / 8.0, f["ln2_g"], f["ln2_b"])
    w1g = np.ascontiguousarray(f["lnf_g"][:, None] * f["ffn_w1"])
    common["w_ff1"] = _wshuf(w1g, FC, DC)
    common["w_sa_o"] = _wshuf(f["sa_wo"], DC, DC)
    common["w_ca_k"] = _wshuf(f["ca_wk"], DC, DC)
    common["w_ca_v"] = _wshuf_v(f["ca_wv"])
    common["w_ca_o"] = _wshuf(f["ca_wo"], DC, DC)
    common["w_ff2"] = _wshuf(f["ffn_w2"], DC, FC)
    common["b_ff1"] = _col(f["ffn_b1"] + f["lnf_b"] @ f["ffn_w1"], 32)
    common["b_ff2"] = _col(f["ffn_b2"], 8)
    sel = np.zeros((2, 128), np.float32)
    sel[0, 0:64] = 1.0
    sel[1, 64:128] = 1.0
    common["sel01"] = sel.astype(bf16)

    in_maps = []
    for core in range(N_CORES):
        b, hh = core // 2, core % 2
        perm = _perm(hh)
        m = dict(common)
        decT = np.ascontiguousarray(f["dec_inputs"][b].T)
        m["xoT_f32"] = np.ascontiguousarray(decT[:, perm])
        m["xoT_bf"] = m["xoT_f32"].astype(bf16)
        m["xaT_bf"] = decT.astype(bf16)
        m["encT_bf"] = np.ascontiguousarray(f["enc_outputs"][b].T).astype(bf16)
        keep = ~f["self_attn_mask"][b]          # [q, k], True = attend
        maskd = np.empty((128, 8, 64), np.float32)
        for s in range(8):
            qg = 64 * (2 * s + hh) + np.arange(64)
            maskd[:, s, :] = keep[qg][:, 128 * s:128 * (s + 1)].T
        m["maskd_bf"] = np.ascontiguousarray(maskd.reshape(128, -1)).astype(bf16)
        in_maps.append(m)
    return in_maps


def run(inputs, trace=False):
    nc, _ = _build()
    in_maps = _prep_in_maps(inputs)
    res = run_bass_kernel_spmd(nc, in_maps, list(range(N_CORES)), trace=trace)
    x = np.empty((B, L, D), np.float32)
    sa = np.empty((B, L, L), np.float32)
    ca = np.empty((B, L, L), np.float32)
    keep_all = ~np.asarray(inputs["self_attn_mask"])
    for core in range(N_CORES):
        b, hh = core // 2, core % 2
        perm = _perm(hh)
        r = res.results[core]
        x[b, perm, :] = r["outT"].T
        sa_p = r["sa_top"].astype(np.float32).T / r["sa_topd"][0][:, None]
        sa[b, perm, :] = np.where(keep_all[b][perm], sa_p, 0.0)
        ca[b, perm, :] = (r["ca_top"].astype(np.float32).T
                          / r["ca_topd"][0][:, None])
    return (x, sa, ca), res


def kernel(**inputs):
    out, _ = run(inputs, trace=False)
    return out
